# revision 58
# baseline (speedup 1.0000x reference)
"""MultiHeadInfiniAttention Trainium2 kernel (8 NeuronCores).

Problem: B=2, T=4096, D=1024, H=8 heads x 128 dh, SEG_LEN=512 (8 segments).
Per (b,h): segment-recurrent memory (M||z||z', bf16 chain) + local causal
softmax attention, gated combine.

Sharding: 16 (b,h) pairs over 8 cores -> core c handles b=c//4 and heads
{2*(c%4), 2*(c%4)+1}.

fp8 projection scheme (DoubleRow matmuls, 0.5 cyc/row, 2 k-tiles/instr):
  host: x = xh(e4m3) + xl(e5m2 residual); W' = 64*W -> wh(e4m3),
  wl(e5m2 residual); the 1/64 folds into the ACT evacuation scales.
  q,k ("xcomp"): q = wh^T(xh + xl)    [w-quant err ~1.3%]
  v  (half-comp, natural layout): v = (wh+wl)^T xh + wh^T xl  [~exact]
Scores / a_dot / memory matmuls run in bf16.  Gate is applied via
scaled-ones columns (z' = z/g chain; pden rhs = 1/(1-g)) so no per-core
constants are baked (SPMD-safe).  v-bias is added host-side (it commutes
through the recurrence exactly).  Output is stored bf16.
"""

import os
import sys

sys.path.insert(0, os.path.dirname(os.path.abspath(__file__)))

import numpy as np
import ml_dtypes

import concourse.bass as bass
import concourse.mybir as mybir
import concourse.tile as tile
from concourse import bass_utils
from concourse.bass import ts


def split_multi_waits(nc, max_waits: int = 1) -> int:
    """This container's walrus build only supports ONE sync wait per
    instruction.  Tile emits multi-wait instructions; split the extras onto
    same-engine NOP carriers inserted right before each instruction."""
    n_split = 0
    for func in nc.m.functions:
        for bb in func.blocks:
            insts = bb.instructions
            new_list = []
            changed = False
            for inst in insts:
                si = inst.sync_info
                if si is not None and si.on_wait and len(si.on_wait) > max_waits:
                    waits = list(si.on_wait)
                    for w in waits[max_waits:]:
                        nop = mybir.InstNoOp(name=f"WSPLIT-{nc.next_id()}")
                        nop.engine = inst.engine
                        nop.sync_info = mybir.SyncInfo(on_wait=[w], on_update=[])
                        new_list.append(nop)
                        n_split += 1
                    inst.sync_info = mybir.SyncInfo(
                        on_wait=waits[:max_waits],
                        on_update=list(si.on_update or []),
                    )
                    changed = True
                new_list.append(inst)
            if changed:
                bb.instructions = new_list
    return n_split


F32 = mybir.dt.float32
BF16 = mybir.dt.bfloat16
E4 = mybir.dt.float8e4
E5 = mybir.dt.float8e5
AF = mybir.ActivationFunctionType
ALU = mybir.AluOpType
DR = mybir.MatmulPerfMode.DoubleRow

B, T, D = 2, 4096, 1024
H, DH, SEG = 8, 128, 512
S = T // SEG          # 8 segments
NCH = D // 128        # 8 contraction chunks
INV_SQRT_D = 1.0 / float(np.sqrt(DH))
MASK_NEG = -1.0e9
WS = 64.0             # host W prescale (fp8 range); 1/WS folds into evacs

LAST_RESULTS = None  # BassKernelResults of the last run (for test.py)


def _build_program(vbias=False):
    nc = bass.Bass("TRN2", target_bir_lowering=False, debug=False)

    xh8 = nc.dram_tensor("xh8", (D, T), E4, kind="ExternalInput")
    xl8 = nc.dram_tensor("xl8", (D, T), E5, kind="ExternalInput")
    # weights pre-swizzled host-side to the SBUF image [128, NCH*256] so the
    # load is one DMA with 2KB contiguous runs per partition
    whq = nc.dram_tensor("whq", (128, NCH * 2 * DH), E4, kind="ExternalInput")
    whk = nc.dram_tensor("whk", (128, NCH * 2 * DH), E4, kind="ExternalInput")
    whv = nc.dram_tensor("whv", (128, NCH * 2 * DH), E4, kind="ExternalInput")
    wlv = nc.dram_tensor("wlv", (128, NCH * 2 * DH), E5, kind="ExternalInput")
    biases = nc.dram_tensor("biases", (128, 4), F32, kind="ExternalInput")
    vbrow = (nc.dram_tensor("vbrow", (1, 2 * DH), BF16, kind="ExternalInput")
             if vbias else None)
    # gates | ident | tril packed: one bf16 const DMA
    cbf16_d = nc.dram_tensor("cbf16", (128, 6 + 2 * 128), BF16,
                             kind="ExternalInput")
    y = nc.dram_tensor("out", (T, 2 * DH), BF16, kind="ExternalOutput")
    y2 = nc.dram_tensor("out2", (T, 2 * DH), BF16, kind="ExternalOutput")

    with tile.TileContext(nc) as tc:
        _emit(nc, tc, xh8, xl8, whq, whk, whv, wlv, biases, cbf16_d,
              vbrow, y, y2)

    split_multi_waits(nc)
    return nc


def _emit(nc, tc, xh8, xl8, whq, whk, whv, wlv, biases, cbf16_d, vbrow, y,
          y2):
    from contextlib import ExitStack

    ctx = ExitStack()
    with ctx:
        singles = ctx.enter_context(tc.tile_pool(name="singles", bufs=1))
        state = ctx.enter_context(tc.tile_pool(name="state", bufs=2))
        xpool = ctx.enter_context(tc.tile_pool(name="xts", bufs=3))
        work = ctx.enter_context(tc.tile_pool(name="work", bufs=4))
        small = ctx.enter_context(tc.tile_pool(name="small", bufs=8))
        outp = ctx.enter_context(tc.tile_pool(name="outp", bufs=2))
        # PSUM pools -- exactly 8 banks
        proj_ps = ctx.enter_context(tc.tile_pool(name="proj_ps", bufs=3, space="PSUM"))
        sc_ps = ctx.enter_context(tc.tile_pool(name="sc_ps", bufs=2, space="PSUM"))
        mem_ps = ctx.enter_context(tc.tile_pool(name="mem_ps", bufs=2, space="PSUM"))
        ucd_ps = ctx.enter_context(tc.tile_pool(name="ucd_ps", bufs=1, space="PSUM"))

        # ---- weights: [128, NCH, 256] per matrix ----
        w_sb = {}
        for name, dram in (("whq", whq), ("whk", whk), ("whv", whv)):
            w_sb[name] = singles.tile([128, NCH, 2 * DH], E4, tag=f"w_{name}",
                                      name=f"w_{name}")
        w_sb["wlv"] = singles.tile([128, NCH, 2 * DH], E5, tag="w_wlv",
                                   name="w_wlv")
        w_views = {"whq": whq.ap(), "whk": whk.ap(),
                   "whv": whv.ap(), "wlv": wlv.ap()}

        # ---- persistent per-head state: M || z || z'  (bf16, [128, 130]) ---
        mz_bf = []
        for hi in range(2):
            mz_bf.append([
                state.tile([128, DH + 2], BF16, tag="mz_bf", bufs=4,
                           name=f"mzb_{hi}_{k}")
                for k in range(2)
            ])
        # persistent v_ones rings (per head); gate cols written once
        vo_ring = [
            [state.tile([128, 4, DH + 2], BF16, tag="vo_ring", bufs=4,
                        name=f"vo_{hi}_{k}") for k in range(2)]
            for hi in range(2)
        ]

        yv = y.ap().rearrange("(s tile p) (h e) -> s p tile h e",
                              p=128, tile=4, h=2)
        y2v = y2.ap().rearrange("(s tile p) (h e) -> s p tile h e",
                                p=128, tile=4, h=2)
        xhv = xh8.ap().rearrange("(c p) t -> p c t", p=128)
        xlv = xl8.ap().rearrange("(c p) t -> p c t", p=128)

        def load_slab(s, split):
            sh = xpool.tile([128, NCH, SEG], E4, tag="xh", name=f"xh{s}")
            sl = xpool.tile([128, NCH, SEG], E5, tag="xl", name=f"xl{s}")
            if split:
                for g in range(4):
                    nc.sync.dma_start(out=sh[:, 2 * g:2 * g + 2, :],
                                      in_=xhv[:, 2 * g:2 * g + 2, ts(s, SEG)])
                for g in range(4):
                    nc.sync.dma_start(out=sl[:, 2 * g:2 * g + 2, :],
                                      in_=xlv[:, 2 * g:2 * g + 2, ts(s, SEG)])
            else:
                nc.sync.dma_start(out=sh[:], in_=xhv[:, :, ts(s, SEG)])
                nc.sync.dma_start(out=sl[:], in_=xlv[:, :, ts(s, SEG)])
            return sh, sl

        # startup: few large DMAs, ordered so the q projection unblocks first
        slab0h = xpool.tile([128, NCH, SEG], E4, tag="xh", name="xh0")
        slab0l = xpool.tile([128, NCH, SEG], E5, tag="xl", name="xl0")
        bias_sb = singles.tile([128, 4], F32, tag="bias")
        cbf16 = singles.tile([128, 6 + 2 * 128], BF16, tag="cbf16")
        nc.sync.dma_start(out=w_sb["whq"][:], in_=w_views["whq"])
        nc.sync.dma_start(out=slab0h[:], in_=xhv[:, :, ts(0, SEG)])
        nc.sync.dma_start(out=slab0l[:], in_=xlv[:, :, ts(0, SEG)])
        nc.sync.dma_start(out=bias_sb[:], in_=biases.ap())
        nc.sync.dma_start(out=w_sb["whk"][:], in_=w_views["whk"])
        nc.sync.dma_start(out=cbf16[:], in_=cbf16_d.ap())
        nc.sync.dma_start(out=w_sb["whv"][:], in_=w_views["whv"])
        nc.sync.dma_start(out=w_sb["wlv"][:], in_=w_views["wlv"])
        gate_sb = cbf16[:, 0:6]
        ident = cbf16[:, 6:134]
        trilm = cbf16[:, 134:262]

        # fill v_ones gate columns once: (-1, 1/g_hi)
        for hi in range(2):
            for k in range(2):
                gcols = bass.AP(
                    tensor=gate_sb.tensor, offset=gate_sb.offset,
                    ap=[gate_sb.ap[0], [0, 4], [1 + hi if hi else 1, 2]],
                )
                nc.vector.tensor_copy(vo_ring[hi][k][:, :, DH:DH + 2], gcols)

        consts = dict(bias=bias_sb, gate=gate_sb, ident=ident, trilm=trilm)
        if vbrow is not None:
            # v-bias path: ones row (K=1 lhsT) and 64*b_v row for the
            # rank-1 bias add into the v projection psum
            vb_sb = singles.tile([1, 2 * DH], BF16, tag="vb")
            nc.sync.dma_start(out=vb_sb[:], in_=vbrow.ap())
            onesrow = singles.tile([1, 128], BF16, tag="onesrow")
            nc.gpsimd.memset(onesrow[:], 1.0)
            consts["vb"] = vb_sb
            consts["onesrow"] = onesrow
        pools = dict(work=work, small=small, proj=proj_ps, sc=sc_ps,
                     mem=mem_ps, ucd=ucd_ps)

        slabs = [None] * S
        slabs[0] = (slab0h, slab0l)

        pr = [[None, None] for _ in range(S)]
        pr[0][0] = _produce(nc, 0, 0, slabs[0], w_sb, consts, pools,
                            vo_ring[0][0])
        pr[0][1] = _produce(nc, 0, 1, slabs[0], w_sb, consts, pools,
                            vo_ring[1][0])
        slabs[1] = load_slab(1, split=False)

        for s in range(S):
            if s + 2 < S:
                slabs[s + 2] = load_slab(s + 2, split=False)
            # one shared per-segment bank: head hi's uc/dens at col 256*hi
            ucd = ucd_ps.tile([128, 512], F32, tag="ucd", name=f"ucd_{s}")
            mzp = [mz_bf[0][(s - 1) % 2], mz_bf[1][(s - 1) % 2]]
            sc1 = _scan_early2(nc, s, pr[s], consts, pools, ucd, mzp, y2v)
            # --- per head: M update first (chain DVE ops queue early), then
            # produce(s+1) to fill the PE; h0's combine is emitted before
            # produce(h1) so its adot-psum readers run early ---
            _scan_mem(nc, s, 0, pr[s][0], sc1[0], pools,
                      mzp[0], mz_bf[0][s % 2])
            if s + 1 < S:
                pr[s + 1][0] = _produce(nc, s + 1, 0, slabs[s + 1], w_sb,
                                        consts, pools, vo_ring[0][(s + 1) % 2])
            _scan_mem(nc, s, 1, pr[s][1], sc1[1], pools,
                      mzp[1], mz_bf[1][s % 2])
            _scan_out(nc, s, 0, pr[s][0], sc1[0], consts, pools, yv)
            if s + 1 < S:
                pr[s + 1][1] = _produce(nc, s + 1, 1, slabs[s + 1], w_sb,
                                        consts, pools, vo_ring[1][(s + 1) % 2])
            _scan_out(nc, s, 1, pr[s][1], sc1[1], consts, pools, yv)


def _produce(nc, s, hi, slabs, w_sb, consts, pools, v_ones):
    """Projections (fp8 DoubleRow), evacuations, elu, natural-v, sk^T."""
    xh, xl = slabs
    work, small = pools["work"], pools["small"]
    proj_ps = pools["proj"]
    bias_sb, gate_sb, ident = consts["bias"], consts["gate"], consts["ident"]
    hc = ts(hi, DH)     # this head's weight columns

    out = {}

    def project_qk(wname, bcol, tag):
        ps = proj_ps.tile([128, SEG], F32, tag="proj",
                          name=f"p{tag}_{s}_{hi}")
        w = w_sb[wname]
        for g in range(4):
            nc.tensor.matmul(
                ps[:], w[:, 2 * g:2 * g + 2, hc], xh[:, 2 * g:2 * g + 2, :],
                start=(g == 0), stop=False, perf_mode=DR,
                skip_group_check=True,
            )
        for g in range(4):
            nc.tensor.matmul(
                ps[:], w[:, 2 * g:2 * g + 2, hc], xl[:, 2 * g:2 * g + 2, :],
                start=False, stop=(g == 3), perf_mode=DR,
                skip_group_check=True,
            )
        bf = work.tile([128, SEG], BF16, tag=f"{tag}_bf", bufs=4,
                       name=f"{tag}bf_{s}_{hi}")
        nc.scalar.activation(bf[:], ps[:], AF.Identity,
                             bias=bias_sb[:, bcol:bcol + 1], scale=1.0 / WS)
        return ps, bf

    # ---- q ----
    qt_ps, q_bf = project_qk("whq", 0 + hi, "q")
    if s > 0:
        exq = work.tile([128, SEG], BF16, tag="exq", bufs=2,
                        name=f"exq_{s}_{hi}")
        nc.scalar.activation(exq[:], qt_ps[:], AF.Exp,
                             bias=bias_sb[:, 0 + hi:1 + hi], scale=1.0 / WS)
        sq = work.tile([128, SEG], BF16, tag="sq", bufs=3,
                       name=f"sq_{s}_{hi}")
        # elu(x)+1 = min(exp(x), 1+x)
        nc.vector.scalar_tensor_tensor(
            out=sq[:], in0=q_bf[:], scalar=1.0, in1=exq[:],
            op0=ALU.add, op1=ALU.min,
        )
        out["sq"] = sq
    # ---- k ----
    kt_ps, k_bf = project_qk("whk", 2 + hi, "k")
    if s < S - 1:
        exk = work.tile([128, SEG], BF16, tag="exk", bufs=2,
                        name=f"exk_{s}_{hi}")
        nc.scalar.activation(exk[:], kt_ps[:], AF.Exp,
                             bias=bias_sb[:, 2 + hi:3 + hi], scale=1.0 / WS)
        sk = work.tile([128, SEG], BF16, tag="sk", bufs=3,
                       name=f"sk_{s}_{hi}")
        nc.vector.scalar_tensor_tensor(
            out=sk[:], in0=k_bf[:], scalar=1.0, in1=exk[:],
            op0=ALU.add, op1=ALU.min,
        )
        out["sk"] = sk

    # ---- v: natural layout [t, dh], half-comp fp8 ----
    # terms: wh^T xh (A: lhsT=xh pairs e4, rhs=wh pairs e4),
    #        wh^T xl (B: lhsT=xl pairs e5, rhs=wh e4),
    #        wl^T xh (C: lhsT=xh pairs e4, rhs=wl e5)
    vps = proj_ps.tile([128, 4, DH], F32, tag="proj", name=f"pv_{s}_{hi}")
    whv, wlv = w_sb["whv"], w_sb["wlv"]
    first = True
    for j in range(4):
        tsl = ts(j, 128)
        for g in range(4):
            cp = slice(2 * g, 2 * g + 2)
            nc.tensor.matmul(
                vps[:, j, :], xh[:, cp, tsl], whv[:, cp, hc],
                start=first, stop=False, perf_mode=DR, skip_group_check=True,
            )
            first = False
        for g in range(4):
            cp = slice(2 * g, 2 * g + 2)
            nc.tensor.matmul(
                vps[:, j, :], xl[:, cp, tsl], whv[:, cp, hc],
                start=False, stop=False, perf_mode=DR, skip_group_check=True,
            )
        for g in range(4):
            cp = slice(2 * g, 2 * g + 2)
            nc.tensor.matmul(
                vps[:, j, :], xh[:, cp, tsl], wlv[:, cp, hc],
                start=False, stop=(g == 3 and "vb" not in consts),
                perf_mode=DR, skip_group_check=True,
            )
        if "vb" in consts:
            # + ones^T (64*b_v): exact on-device v bias
            nc.tensor.matmul(
                vps[:, j, :], consts["onesrow"][:], consts["vb"][:, hc],
                start=False, stop=True, skip_group_check=True,
            )
    # v_ones [128, 4, 130]: v | -1 | 1/g  (gate cols persist in the ring)
    nc.scalar.activation(v_ones[:, :, :DH], vps[:], AF.Copy, scale=1.0 / WS)

    out.update(q_bf=q_bf, k_bf=k_bf, v_ones=v_ones)
    return out


def _scan_early2(nc, s, prs, consts, pools, ucd, mzp, y2v):
    """Both heads' retr/dens, scores+exps, uc-v matmuls, retr_n / amem_cat,
    interleaved so psum-ring WAR waits are covered by PE work."""
    work, small = pools["work"], pools["small"]
    sc_ps, mem_ps = pools["sc"], pools["mem"]
    sts = [{"ucd": ucd, "b0": 256 * hi} for hi in range(2)]

    # ---- sk natural via PE transpose (input ready since last segment) ----
    if s < S - 1:
        for hi in range(2):
            skt_ps = pools["proj"].tile([128, 4, 128], BF16, tag="proj",
                                        name=f"skt_{s}_{hi}")
            for i in range(4):
                nc.tensor.transpose(skt_ps[:, i, :],
                                    prs[hi]["sk"][:, ts(i, 128)],
                                    consts["ident"][:])
            sk_nat = work.tile([128, 4, 128], BF16, tag="sk_nat", bufs=2,
                               name=f"sknat_{s}_{hi}")
            nc.vector.tensor_copy(sk_nat[:], skt_ps[:])
            prs[hi]["sk_nat"] = sk_nat

    # ---- retr + den_k, h0 then h1 ----
    for hi in range(2):
        if not 0 < s < S - 1:
            continue
        sk, b0 = prs[hi]["sk"], sts[hi]["b0"]
        rps = mem_ps.tile([128, 4, DH], F32, tag="mem", name=f"retr_{s}_{hi}")
        for c in range(4):
            nc.tensor.matmul(
                rps[:, c, :], sk[:, ts(c, 128)], mzp[hi][:, :DH],
                start=(c == 0), stop=(c == 3), skip_group_check=True,
            )
            nc.tensor.matmul(
                ucd[:, b0 + 130 + c:b0 + 131 + c], sk[:, ts(c, 128)],
                mzp[hi][:, DH:DH + 1],
                start=(c == 0 and hi == 0), stop=True, skip_group_check=True,
            )
        sts[hi]["rps"] = rps

    def scores01(hi):
        q_bf, k_bf = prs[hi]["q_bf"], prs[hi]["k_bf"]
        ptj = []
        for j in range(2):
            t_cols = (4 - j) * 128
            sc = sc_ps.tile([128, SEG], F32, tag="scores",
                            name=f"sc_{s}_{hi}_{j}")
            nc.tensor.matmul(
                sc[:, :t_cols], k_bf[:, ts(j, 128)], q_bf[:, j * 128:],
                start=True, stop=True, skip_group_check=True,
            )
            pt = work.tile([128, t_cols], BF16, tag=f"pt{j}", bufs=2,
                           name=f"pt{j}_{s}_{hi}")
            nc.scalar.activation(pt[:], sc[:, :t_cols], AF.Exp,
                                 scale=INV_SQRT_D)
            # zero the upper triangle of the diagonal block (causal mask)
            nc.vector.tensor_mul(pt[:, :128], pt[:, :128], consts["trilm"][:])
            ptj.append(pt)
        sts[hi]["ptj"] = ptj

    def scores23(hi):
        # cols [0:256] = j2 (t 256:512), [256:384] = j3 (t 384:512)
        q_bf, k_bf = prs[hi]["q_bf"], prs[hi]["k_bf"]
        sc23 = sc_ps.tile([128, 384], F32, tag="scores",
                          name=f"sc_{s}_{hi}_23")
        nc.tensor.matmul(
            sc23[:, 0:256], k_bf[:, ts(2, 128)], q_bf[:, 256:],
            start=True, stop=False, skip_group_check=True,
        )
        nc.tensor.matmul(
            sc23[:, 256:384], k_bf[:, ts(3, 128)], q_bf[:, 384:],
            start=False, stop=True, skip_group_check=True,
        )
        pt23 = work.tile([128, 384], BF16, tag="pt23", bufs=2,
                         name=f"pt23_{s}_{hi}")
        nc.scalar.activation(pt23[:], sc23[:], AF.Exp, scale=INV_SQRT_D)
        # mask both diagonal blocks (cols 0:128 = j2 diag, 256:384 = j3 diag)
        dg = bass.AP(tensor=pt23.tensor, offset=pt23.offset,
                     ap=[pt23.ap[0], [256, 2], [1, 128]])
        trilb = consts["trilm"]
        tril2 = bass.AP(tensor=trilb.tensor, offset=trilb.offset,
                        ap=[trilb.ap[0], [0, 2], [1, 128]])
        nc.vector.tensor_mul(dg, dg, tril2)
        sts[hi]["ptj"].append(pt23)

    def retr_n(hi):
        if not 0 < s < S - 1:
            return
        b0 = sts[hi]["b0"]
        rkn = small.tile([128, 4], F32, tag="rkn", name=f"rkn_{s}_{hi}")
        nc.vector.reciprocal(rkn[:], ucd[:, b0 + 130:b0 + 134])
        rkn_bc = bass.AP(
            tensor=rkn.tensor, offset=rkn.offset,
            ap=[rkn.ap[0], rkn.ap[1], [0, 128]],
        )
        rn = work.tile([128, 4, 128], BF16, tag="retr_n", bufs=2,
                       name=f"rn_{s}_{hi}")
        nc.vector.tensor_mul(rn[:], sts[hi]["rps"][:], rkn_bc)
        sts[hi]["retr_n"] = rn

    def ucv(hi):
        # uc v-part: ready early, used as PE filler between score tiles
        if s >= S - 1:
            return
        v_ones, sk_nat = prs[hi]["v_ones"], prs[hi]["sk_nat"]
        b0 = sts[hi]["b0"]
        for j in range(4):
            nc.tensor.matmul(
                ucd[:, b0:b0 + DH + 2], sk_nat[:, j, :], v_ones[:, j, :],
                start=(s == 0 and j == 0 and hi == 0),
                stop=(s == 0 and j == 3),
                skip_group_check=True,
            )

    def amem(hi):
        if s == 0:
            return
        sq, b0 = prs[hi]["sq"], sts[hi]["b0"]
        aps = mem_ps.tile([128, 4, DH], F32, tag="mem", name=f"amem_{s}_{hi}")
        for c in range(4):
            nc.tensor.matmul(
                aps[:, c, :], sq[:, ts(c, 128)], mzp[hi][:, :DH],
                start=(c == 0), stop=(c == 3), skip_group_check=True,
            )
            # aden vs z' = z/g  ->  recip gives g/(sq.z)
            nc.tensor.matmul(
                ucd[:, b0 + 134 + c:b0 + 135 + c], sq[:, ts(c, 128)],
                mzp[hi][:, DH + 1:DH + 2],
                start=(s == S - 1 and c == 0 and hi == 0), stop=True,
                skip_group_check=True,
            )
        sts[hi]["aps"] = aps

    def amem_cat(hi):
        if s == 0:
            return
        b0 = sts[hi]["b0"]
        rg = small.tile([128, 4], F32, tag="rg", name=f"rg_{s}_{hi}")
        nc.vector.reciprocal(rg[:], ucd[:, b0 + 134:b0 + 138])
        rg_bc = bass.AP(
            tensor=rg.tensor, offset=rg.offset,
            ap=[rg.ap[0], rg.ap[1], [0, 128]],
        )
        ac = work.tile([128, 4, 128], BF16, tag="amem_cat", bufs=2,
                       name=f"ac_{s}_{hi}")
        nc.vector.tensor_mul(ac[:], sts[hi]["aps"][:], rg_bc)
        nc.sync.dma_start(out=y2v[s, :, :, hi], in_=ac[:])

    scores01(0)
    retr_n(0)
    ucv(0)
    scores01(1)
    retr_n(1)
    ucv(1)
    amem(0)
    scores23(0)
    amem(1)
    scores23(1)
    amem_cat(0)
    amem_cat(1)
    return sts


def _scan_mem(nc, s, hi, pr, st, pools, mzb_prev, mzb_new):
    """uc2 matmuls + M||z||z' chain update."""
    sk_nat = pr.get("sk_nat")
    ucd, b0 = st["ucd"], st["b0"]
    if s >= S - 1:
        return
    if s > 0:
        for j in range(4):
            nc.tensor.matmul(
                ucd[:, b0:b0 + DH], sk_nat[:, j, :], st["retr_n"][:, j, :],
                start=False, stop=(j == 3), skip_group_check=True,
            )
    if s == 0:
        nc.vector.tensor_copy(mzb_new[:], ucd[:, b0:b0 + DH + 2])
    else:
        nc.vector.tensor_add(mzb_new[:], ucd[:, b0:b0 + DH + 2], mzb_prev[:])


def _scan_out(nc, s, hi, pr, st, consts, pools, yv):
    """a_dot + pden matmuls, gated a_dot term (amem term stored separately;
    the host adds the two)."""
    work, small = pools["work"], pools["small"]
    mem_ps = pools["mem"]
    gate_sb = consts["gate"]
    v_ones = pr["v_ones"]
    ptj, ucd, b0 = st["ptj"], st["ucd"], st["b0"]

    # ---- a_dot: adot[t-block i] = sum_j P^T_j(i)^T @ v_j ; pden vs 1/(1-g)
    adot = mem_ps.tile([128, 4, DH], F32, tag="mem", name=f"adot_{s}_{hi}")
    pcol = bass.AP(
        tensor=gate_sb.tensor, offset=gate_sb.offset + 3 + hi,
        ap=[gate_sb.ap[0], [1, 1]],
    )
    for j in range(4):
        src = ptj[min(j, 2)]
        for i in range(j, 4):
            lo = (i - j) * 128 + (256 if j == 3 else 0)
            nc.tensor.matmul(
                adot[:, i, :], src[:, lo:lo + 128], v_ones[:, j, :DH],
                start=(j == 0 and i == 0), stop=(j == i),
                skip_group_check=True,
            )
            nc.tensor.matmul(
                ucd[:, b0 + 138 + i:b0 + 139 + i], src[:, lo:lo + 128], pcol,
                start=False, stop=(j == i), skip_group_check=True,
            )

    # ---- gated a_dot term -> y1 ----
    rdot = small.tile([128, 4], F32, tag="rdot", name=f"rdot_{s}_{hi}")
    nc.vector.reciprocal(rdot[:], ucd[:, b0 + 138:b0 + 142])
    rdot_bc = bass.AP(
        tensor=rdot.tensor, offset=rdot.offset,
        ap=[rdot.ap[0], rdot.ap[1], [0, 128]],
    )
    tmp = work.tile([128, 4, 128], BF16, tag="a_tmp", bufs=2,
                    name=f"tmp_{s}_{hi}")
    nc.vector.tensor_mul(tmp[:], adot[:], rdot_bc)
    nc.sync.dma_start(out=yv[s, :, :, hi], in_=tmp[:])


_NC_CACHE = {}


def _get_nc(vbias=False):
    if vbias not in _NC_CACHE:
        _NC_CACHE[vbias] = _build_program(vbias)
    return _NC_CACHE[vbias]


def _host_consts():
    ident = np.eye(128, dtype=ml_dtypes.bfloat16)
    # trilm[m, t] (P^T layout): keep m <= t within the diagonal block
    trilm = np.triu(np.ones((128, 128), np.float32)).astype(ml_dtypes.bfloat16)
    return ident, trilm


def kernel(x, w_q, b_q, w_k, b_k, w_v, b_v, beta, _trace=False):
    global LAST_RESULTS
    x = np.asarray(x, dtype=np.float32)
    w_q = np.asarray(w_q, dtype=np.float32)
    b_q = np.asarray(b_q, dtype=np.float32)
    w_k = np.asarray(w_k, dtype=np.float32)
    b_k = np.asarray(b_k, dtype=np.float32)
    w_v = np.asarray(w_v, dtype=np.float32)
    b_v = np.asarray(b_v, dtype=np.float32)
    beta = np.asarray(beta, dtype=np.float32)

    gate = 1.0 / (1.0 + np.exp(-beta))  # sigmoid, [H]
    vbias = bool(np.any(b_v))
    ident, trilm = _host_consts()

    # per-batch fp8 decomposition of x^T (shared by 4 cores each)
    xh_b, xl_b = [], []
    for b in range(B):
        xt = np.ascontiguousarray(x[b].T)
        xh = xt.astype(ml_dtypes.float8_e4m3)
        xl = (xt - xh.astype(np.float32)).astype(ml_dtypes.float8_e5m2)
        xh_b.append(xh)
        xl_b.append(xl)

    in_maps = []
    for c in range(8):
        b = c // 4
        h0 = (c % 4) * 2
        cols = slice(h0 * DH, (h0 + 2) * DH)
        def img(a):
            # [D, 256] -> SBUF image [128, NCH*256]
            return np.ascontiguousarray(
                a.reshape(NCH, 128, 2 * DH).transpose(1, 0, 2)
                .reshape(128, NCH * 2 * DH))

        wq64 = (WS * w_q[:, cols])
        wk64 = (WS * w_k[:, cols])
        wv64 = (WS * w_v[:, cols])
        whv_ = wv64.astype(ml_dtypes.float8_e4m3)
        wlv_ = (wv64 - whv_.astype(np.float32)).astype(ml_dtypes.float8_e5m2)
        bias_cols = np.stack(
            [
                b_q[h0 * DH:(h0 + 1) * DH], b_q[(h0 + 1) * DH:(h0 + 2) * DH],
                b_k[h0 * DH:(h0 + 1) * DH], b_k[(h0 + 1) * DH:(h0 + 2) * DH],
            ],
            axis=1,
        ).astype(np.float32)  # [128, 4]
        g0, g1 = float(gate[h0]), float(gate[h0 + 1])
        # col0 = -1: the z column is chained negated so the delta-rule's
        # -retr/(sk.z) needs no separate negation on DVE
        gates_np = np.tile(
            np.array([-1.0, 1.0 / g0, 1.0 / g1,
                      1.0 / (1.0 - g0), 1.0 / (1.0 - g1), 0.0], np.float32),
            (128, 1),
        ).astype(ml_dtypes.bfloat16)
        cbf16 = np.concatenate([gates_np, ident, trilm], axis=1)
        im = {
            "xh8": xh_b[b],
            "xl8": xl_b[b],
            "whq": img(wq64.astype(ml_dtypes.float8_e4m3)),
            "whk": img(wk64.astype(ml_dtypes.float8_e4m3)),
            "whv": img(whv_),
            "wlv": img(wlv_),
            "biases": np.ascontiguousarray(bias_cols),
            "cbf16": np.ascontiguousarray(cbf16),
        }
        if vbias:
            im["vbrow"] = (WS * b_v[None, h0 * DH:(h0 + 2) * DH]).astype(
                ml_dtypes.bfloat16)
        in_maps.append(im)

    nc = _get_nc(vbias)
    LAST_RESULTS = bass_utils.run_bass_kernel_spmd(
        nc, in_maps, core_ids=list(range(8)), trace=_trace
    )

    out = np.empty((B, T, H * DH), np.float32)
    for c in range(8):
        b = c // 4
        h0 = (c % 4) * 2
        yc = LAST_RESULTS.results[c]["out"].astype(np.float32)
        # amem term (segment 0 rows of out2 are never written -> skip them)
        yc[SEG:] += LAST_RESULTS.results[c]["out2"][SEG:].astype(np.float32)
        out[b, :, h0 * DH:(h0 + 2) * DH] = yc
    return out
